# revision 6
# baseline (speedup 1.0000x reference)
"""Balanced CE loss + accuracy on 8 Trainium2 NeuronCores (Bass/Tile).

Reference computation (N = 16777216 elements):
    loss = -sum(where(t==1, 1.6*log(p), 0.4*log(1-p))) / N
    acc  = mean(round(p) == t)

Strategy (data-parallel over N, no collectives needed):
  Shard N across 8 cores; per core stream 2048-column sub-chunks
  ([128, 2048] tiles, 2 MB DMA each) so the DMA pipe never idles.
  Single-variable encoding z = p - t folds both classes into one value:
    t==1 -> z = p-1 in (-1,0),  y := 1-|z| = p
    t==0 -> z = p   in ( 0,1),  y := 1-|z| = 1-p
  so the per-element log term is ln(y) with class weight w = 1.2*t+0.4,
  and "correct" (round(p)==t) is exactly |z| < 0.5  <=>  ln(y) >= -ln2.
  Work is spread so every engine stays under the ~44us DMA shadow:
    DVE : z = p - t (tensor_tensor) ; count mask m = (q >= -ln2) bf16
    ACT : za = Abs(z) ; q = Ln(-za+1) bf16 with fused accum -> S[s]
    PE  : ones^T @ m accumulated into one PSUM bank -> correct count
    POOL: stt (z < 0) * q with fused accum -> S1[s] = sum_{t=1} ln(p)
  Host folds the [128, 2*NSUB+1] partials in f64:
    loss = -(0.4*S + 1.2*S1)/N,  acc = C/N.
"""

import sys

if "/opt/trn_rl_repo" not in sys.path:
    sys.path.insert(0, "/opt/trn_rl_repo")

import numpy as np

import concourse.bass as bass
import concourse.bacc as bacc
import concourse.tile as tile
from concourse import mybir
from concourse.bass_utils import run_bass_kernel_spmd

N_CORES = 8
N = 16777216
P = 128
SHARD = N // N_CORES          # 2097152 elements per core
COLS = SHARD // P             # 16384 columns per core
SUB = 2048                    # sub-chunk columns (8KB/partition DMA rows)
NSUB = COLS // SUB            # 8 sub-chunks
MMCOL = 512                   # matmul free-dim tile (one PSUM bank)

AF = mybir.ActivationFunctionType
OP = mybir.AluOpType
LN2 = 0.6931471805599453

_NC_CACHE = None


def build_bass():
    """Build the single-core Bass program (SPMD across 8 cores)."""
    global _NC_CACHE
    if _NC_CACHE is not None:
        return _NC_CACHE

    nc = bacc.Bacc("TRN2", target_bir_lowering=False, debug=False)

    p_in = nc.dram_tensor("p_in", [SHARD], mybir.dt.float32, kind="ExternalInput").ap()
    t_in = nc.dram_tensor("t_in", [SHARD], mybir.dt.int32, kind="ExternalInput").ap()
    # acc columns: [s] sum ln(y); [NSUB+s] sum_{t=1} ln(p); [2*NSUB] count
    acc = nc.dram_tensor("acc", [P, 2 * NSUB + 1], mybir.dt.float32, kind="ExternalOutput").ap()

    n_mm = COLS // MMCOL

    with tile.TileContext(nc) as tc:
        with (
            tc.tile_pool(name="io", bufs=5) as io_pool,
            tc.tile_pool(name="zp", bufs=3) as z_pool,
            tc.tile_pool(name="qp", bufs=3) as q_pool,
            tc.tile_pool(name="psum", bufs=1, space=bass.MemorySpace.PSUM) as psum_pool,
            tc.tile_pool(name="misc", bufs=1) as misc_pool,
        ):
            ones = misc_pool.tile([P, P], mybir.dt.bfloat16, tag="ones")
            nc.gpsimd.memset(ones[:], 1.0)
            acc_sb = misc_pool.tile([P, 2 * NSUB + 1], mybir.dt.float32, tag="acc")
            junk_s = misc_pool.tile([P, SUB], mybir.dt.bfloat16, tag="js")
            ps = psum_pool.tile([P, MMCOL], mybir.dt.float32, tag="ps")

            mm = 0
            for s in range(NSUB):
                off = s * SUB * P
                p_t = io_pool.tile([P, SUB], mybir.dt.float32, tag="p")
                t_t = io_pool.tile([P, SUB], mybir.dt.int32, tag="t")
                z_t = z_pool.tile([P, SUB], mybir.dt.float32, tag="z")
                za_t = z_pool.tile([P, SUB], mybir.dt.float32, tag="za")
                q_t = q_pool.tile([P, SUB], mybir.dt.bfloat16, tag="q")
                m_t = q_pool.tile([P, SUB], mybir.dt.bfloat16, tag="m")
                # split the first sub-chunk's DMA/compute in half so the
                # pipeline starts earlier
                nhalf = 2 if s == 0 else 1
                hc = SUB // nhalf
                for h in range(nhalf):
                    ho = off + h * hc * P
                    hs = slice(h * hc, (h + 1) * hc)
                    nc.sync.dma_start(
                        p_t[:, hs], p_in[ho : ho + hc * P].rearrange("(p f) -> p f", p=P)
                    )
                    nc.sync.dma_start(
                        t_t[:, hs], t_in[ho : ho + hc * P].rearrange("(p f) -> p f", p=P)
                    )
                    # z = p - t
                    nc.vector.tensor_tensor(z_t[:, hs], p_t[:, hs], t_t[:, hs], OP.subtract)
                # z <- |z| (in place; same act table as Ln)
                nc.scalar.activation(za_t[:], z_t[:], AF.Abs)
                # q = ln(1 - |z|) = ln(y); accum -> S[s]
                nc.scalar.activation(q_t[:], za_t[:], AF.Ln, bias=1.0, scale=-1.0,
                                     accum_out=acc_sb[:, s : s + 1])
                # S1[s] = sum_{t=1} q = sum_{t=1} ln(p)
                nc.vector.scalar_tensor_tensor(junk_s[:], t_t[:], 1, q_t[:],
                                               OP.is_ge, OP.mult,
                                               accum_out=acc_sb[:, NSUB + s : NSUB + s + 1])
                # correct-count mask (bf16 in/out -> fast DVE), PE-reduced
                nc.vector.tensor_scalar(m_t[:], q_t[:], -LN2, None, OP.is_ge)
                for j in range(SUB // MMCOL):
                    nc.tensor.matmul(
                        ps[:], ones[:], m_t[:, j * MMCOL : (j + 1) * MMCOL],
                        start=(mm == 0), stop=(mm == n_mm - 1),
                    )
                    mm += 1

            # fold the PSUM count matrix (128 identical rows) into a column
            junk_f = misc_pool.tile([P, MMCOL], mybir.dt.float32, tag="jf")
            nc.vector.tensor_scalar(junk_f[:], ps[:], 1.0 / P, None, OP.mult,
                                    OP.add, accum_out=acc_sb[:, 2 * NSUB : 2 * NSUB + 1])

            nc.sync.dma_start(acc[:], acc_sb[:])

    nc.finalize()
    _NC_CACHE = nc
    return nc


def make_in_maps(input, target):
    inp = np.ascontiguousarray(np.asarray(input, dtype=np.float32)).reshape(
        N_CORES, SHARD
    )
    tgt = np.ascontiguousarray(np.asarray(target, dtype=np.int32)).reshape(
        N_CORES, SHARD
    )
    return [{"p_in": inp[c], "t_in": tgt[c]} for c in range(N_CORES)]


def combine(results):
    """Host-side unshard: reduce the 8 cores' partial sums -> (loss, acc)."""
    S = S1 = C = 0.0
    for r in results:
        aa = np.asarray(r["acc"], dtype=np.float64)
        S += aa[:, 0:NSUB].sum()
        S1 += aa[:, NSUB : 2 * NSUB].sum()
        C += aa[:, 2 * NSUB].sum()
    loss = -(0.4 * S + 1.2 * S1) / N
    acc = C / N
    return np.float32(loss), np.float32(acc)


def run_on_hw(input, target, **spmd_kwargs):
    nc = build_bass()
    in_maps = make_in_maps(input, target)
    return run_bass_kernel_spmd(nc, in_maps, list(range(N_CORES)), **spmd_kwargs)


def kernel(input, target):
    br = run_on_hw(input, target)
    return combine(br.results)
